# revision 1
# baseline (speedup 1.0000x reference)
"""Single-head causal attention (B=4, T=4096, E=1024, H=128) on 8 trn2 cores.

Sharding: core c -> (batch b = c//2, piece p = c%2). Within a batch the 32
query blocks of 128 rows are split even/odd between the two pieces so the
causal workload balances. The device program is identical on all cores
(SPMD); all per-core differences are carried by the input data (gathered
query rows + a causal-boundary mask strip).

Device algorithm (per core, all "transposed" layouts):
  QT = Wq @ xq^T           [H=128, 2048]   (gathered query rows)
  KT = Wk @ x^T            [H=128, 4096]
  VT = Wv @ x^T  -> PE-transpose -> V blocks [128 tok, 128 h]
  per q-tile K (512 queries = in-tile blocks i=0..3):
    for kb in 0..8K+7:   ST[kb] = KT_blk^T @ QT_tile   [128 k, 512 q] (PSUM)
      last 8 kb get an additive causal mask strip (per-core data)
      PT = exp(scale * ST)                             (ACT, PSUM->SBUF)
      OT += V_blk^T @ PT    [128 h, 512 q]             (PSUM accum)
      l  += ones^T @ PT     [1, 512 q]                 (PSUM accum)
    O = (OT / l)^T via PE transpose + per-partition scalar multiply
Matmuls run as float32r (full PE rate for free dim >= 256; fp32 data bits).
"""

import os
import numpy as np

B, T, E, H = 4, 4096, 1024, 128
P = 128
NB_E = E // P           # 8 contraction chunks
TQ = T // 2             # 2048 gathered queries per core
N_QT = TQ // 512        # 4 q-tiles per core
SCALE = float(H) ** -0.5
NEG = -30000.0
N_CORES = 8
F32 = np.float32


def _query_rows(p: int) -> np.ndarray:
    """Absolute row indices of the gathered queries for piece p (in order)."""
    blocks = [np.arange(256 * g + 128 * p, 256 * g + 128 * p + 128) for g in range(16)]
    return np.concatenate(blocks)


def _mask_strip(p: int) -> np.ndarray:
    """maskT [1024 k, 512 q]: 0 where key visible, NEG where masked.

    Row 128*j + kk is in-strip key block j (j=0..7); col 128*i + r is
    in-tile query block i. Visible iff 128*j + kk <= 256*i + 128*p + r.
    """
    kk = np.arange(1024)[:, None]           # 128*j + kk
    qq = np.arange(512)[None, :]
    i, r = qq // 128, qq % 128
    visible = kk <= 256 * i + 128 * p + r
    return np.where(visible, 0.0, NEG).astype(F32)


def _emit(tc, aps):
    import concourse.bass as bass
    from concourse import mybir
    from concourse.masks import make_identity

    nc = tc.nc
    f32 = mybir.dt.float32
    f16 = mybir.dt.float16
    EXP = mybir.ActivationFunctionType.Exp

    xT, xqT, wq, wk, wv, maskT, out = aps

    from contextlib import ExitStack

    ctx = ExitStack()
    with ctx:
        # ---- pools ----
        consts = ctx.enter_context(tc.tile_pool(name="consts", bufs=1))
        x_pool = ctx.enter_context(tc.tile_pool(name="x", bufs=96))
        vt_pool = ctx.enter_context(tc.tile_pool(name="vt", bufs=2))
        pt_pool = ctx.enter_context(tc.tile_pool(name="pt", bufs=4))
        osb_pool = ctx.enter_context(tc.tile_pool(name="osb", bufs=2))
        on_pool = ctx.enter_context(tc.tile_pool(name="on", bufs=4))
        sm_pool = ctx.enter_context(tc.tile_pool(name="sm", bufs=4))
        s_ps = ctx.enter_context(tc.tile_pool(name="sps", bufs=3, space="PSUM"))
        o_ps = ctx.enter_context(tc.tile_pool(name="ops", bufs=2, space="PSUM"))
        l_ps = ctx.enter_context(tc.tile_pool(name="lps", bufs=1, space="PSUM"))
        t_ps = ctx.enter_context(tc.tile_pool(name="tps", bufs=2, space="PSUM"))

        # ---- persistent SBUF tensors ----
        identity = consts.tile([P, P], f32)
        ones = consts.tile([P, 1], f16)
        ones32 = consts.tile([1, 1], f32)
        wq_sb = consts.tile([P, NB_E, P], f16)
        wk_sb = consts.tile([P, NB_E, P], f16)
        wv_sb = consts.tile([P, NB_E, P], f16)
        mask_sb = consts.tile([P, 8, 512], f32)
        kt_all = consts.tile([P, T], f16)
        v_all = consts.tile([P, T // P, P], f16)
        qt_all = consts.tile([P, TQ], f16)

        make_identity(nc, identity[:])
        nc.gpsimd.memset(ones[:], 1.0)
        nc.gpsimd.memset(ones32[:], 1.0)
        nc.sync.dma_start(wq_sb[:], wq.rearrange("(c p) h -> p c h", p=P))
        nc.sync.dma_start(wk_sb[:], wk.rearrange("(c p) h -> p c h", p=P))
        nc.sync.dma_start(wv_sb[:], wv.rearrange("(c p) h -> p c h", p=P))
        nc.sync.dma_start(mask_sb[:], maskT.rearrange("(j p) q -> p j q", p=P))

        def load_x_tiles(src_ap, t0):
            tiles = []
            for c in range(NB_E):
                xt = x_pool.tile([P, 512], f16, tag="x")
                nc.sync.dma_start(xt[:], src_ap[c * P:(c + 1) * P, t0:t0 + 512])
                tiles.append(xt)
            return tiles

        def project(w_sb, x_tiles, dst_ap):
            ps = s_ps.tile([P, 512], f32, tag="sps")
            for c in range(NB_E):
                nc.tensor.matmul(
                    ps[:],
                    lhsT=w_sb[:, c, :],
                    rhs=x_tiles[c][:],
                    start=(c == 0),
                    stop=(c == NB_E - 1),
                )
            nc.vector.tensor_copy(dst_ap, ps[:])
            return ps

        # ---- rounds: interleave projections with attention q-tiles ----
        for tt in range(N_QT):
            # Q projection for q-tile tt
            xq_tiles = load_x_tiles(xqT, tt * 512)
            project(wq_sb, xq_tiles, qt_all[:, tt * 512:(tt + 1) * 512])

            # K/V projections for token tiles 2tt, 2tt+1
            for tok in (2 * tt, 2 * tt + 1):
                xk_tiles = load_x_tiles(xT, tok * 512)
                project(wk_sb, xk_tiles, kt_all[:, tok * 512:(tok + 1) * 512])
                vt = vt_pool.tile([P, 512], f32, tag="vt")
                project(wv_sb, xk_tiles, vt[:])
                for u in range(4):
                    kb = tok * 4 + u
                    tp = t_ps.tile([P, P], f32, tag="tps")
                    nc.tensor.transpose(tp[:], vt[:, u * P:(u + 1) * P], identity[:])
                    nc.vector.tensor_copy(v_all[:, kb, :], tp[:])

            # attention for q-tile tt
            qs = qt_all[:, tt * 512:(tt + 1) * 512]
            ot = o_ps.tile([P, 512], f32, tag="ops")
            lt = l_ps.tile([1, 512], f32, tag="lps")
            nkb = 8 * tt + 8

            s_tiles = [None] * nkb

            def emit_scores(kb):
                if kb < 8 * tt:
                    c0 = 0
                else:
                    j = kb - 8 * tt
                    c0 = P * max(0, -(-(128 * j - 255) // 256))
                s = s_ps.tile([P, 512], f32, tag="sps", name=f"s_{tt}_{kb}")
                nc.tensor.matmul(
                    s[:, c0:512],
                    lhsT=kt_all[:, kb * P:(kb + 1) * P],
                    rhs=qs[:, c0:512],
                    start=True,
                    stop=True,
                )
                s_tiles[kb] = s

            def c0_of(kb):
                if kb < 8 * tt:
                    return 0
                j = kb - 8 * tt
                return P * max(0, -(-(128 * j - 255) // 256))

            emit_scores(0)
            for kb in range(nkb):
                if kb + 1 < nkb:
                    emit_scores(kb + 1)
                s = s_tiles[kb]
                c0 = c0_of(kb)
                if kb >= 8 * tt:
                    # the causal boundary lives in a single 128-col block
                    # (= block c0//128); mask is 0 everywhere right of it
                    j = kb - 8 * tt
                    nc.vector.tensor_add(
                        s[:, c0:c0 + P], s[:, c0:c0 + P],
                        mask_sb[:, j, c0:c0 + P])
                pt = pt_pool.tile([P, 512], f16, tag="pt")
                nc.scalar.activation(pt[:, c0:512], s[:, c0:512], EXP, scale=SCALE)
                nc.tensor.matmul(
                    ot[:, c0:512],
                    lhsT=v_all[:, kb, :],
                    rhs=pt[:, c0:512],
                    start=(kb == 0),
                    stop=(kb == nkb - 1),
                )
                nc.tensor.matmul(
                    lt[:1, c0:512],
                    lhsT=ones[:],
                    rhs=pt[:, c0:512],
                    start=(kb == 0),
                    stop=(kb == nkb - 1),
                )

            # epilogue: normalize + transpose + store
            o_sb = osb_pool.tile([P, 512], f32, tag="osb")
            nc.vector.tensor_copy(o_sb[:], ot[:])
            l_sb = sm_pool.tile([1, 512], f32, tag="lsb")
            nc.vector.tensor_copy(l_sb[:], lt[:])
            on = on_pool.tile([P, 4, P], f32, tag="on")
            for i in range(4):
                lc = t_ps.tile([P, 1], f32, tag="tps", name=f"lc_{tt}_{i}")
                nc.tensor.matmul(
                    lc[:],
                    lhsT=l_sb[:1, i * P:(i + 1) * P],
                    rhs=ones32[:],
                    start=True,
                    stop=True,
                )
                rlc = sm_pool.tile([P, 1], f32, tag="rlc")
                nc.vector.reciprocal(rlc[:], lc[:])
                tp = t_ps.tile([P, P], f32, tag="tps", name=f"otp_{tt}_{i}")
                nc.tensor.transpose(tp[:], o_sb[:, i * P:(i + 1) * P], identity[:])
                nc.vector.tensor_scalar_mul(on[:, i, :], tp[:], rlc[:])
            nc.gpsimd.dma_start(
                out[tt * 512:(tt + 1) * 512, :].rearrange("(i p) h -> p i h", p=P),
                on[:],
            )


def build_program():
    import concourse.tile as tile
    from concourse import bacc, mybir

    f32 = mybir.dt.float32
    f16 = mybir.dt.float16
    nc = bacc.Bacc("TRN2", target_bir_lowering=False, debug=False,
                   num_devices=N_CORES)
    xT = nc.dram_tensor("xT", [E, T], f16, kind="ExternalInput").ap()
    xqT = nc.dram_tensor("xqT", [E, TQ], f16, kind="ExternalInput").ap()
    wq = nc.dram_tensor("wq", [E, H], f16, kind="ExternalInput").ap()
    wk = nc.dram_tensor("wk", [E, H], f16, kind="ExternalInput").ap()
    wv = nc.dram_tensor("wv", [E, H], f16, kind="ExternalInput").ap()
    maskT = nc.dram_tensor("maskT", [1024, 512], f32, kind="ExternalInput").ap()
    out = nc.dram_tensor("out", [TQ, H], f32, kind="ExternalOutput").ap()

    with tile.TileContext(nc) as tc:
        _emit(tc, (xT, xqT, wq, wk, wv, maskT, out))
    nc.compile()
    return nc


def make_in_maps(x, Wq, Wk, Wv):
    """Per-core input maps. x: [B,T,E] f32; W*: [H,E] f32."""
    x = np.asarray(x, dtype=F32)
    wq_t = np.ascontiguousarray(np.asarray(Wq, dtype=F32).T.astype(np.float16))
    wk_t = np.ascontiguousarray(np.asarray(Wk, dtype=F32).T.astype(np.float16))
    wv_t = np.ascontiguousarray(np.asarray(Wv, dtype=F32).T.astype(np.float16))
    in_maps = []
    for c in range(N_CORES):
        b, p = c // 2, c % 2
        xb = x[b]                                              # [T, E]
        xT_np = np.ascontiguousarray(xb.T.astype(np.float16))
        xqT_np = np.ascontiguousarray(xb[_query_rows(p)].T.astype(np.float16))
        in_maps.append({
            "xT": xT_np,
            "xqT": xqT_np,
            "wq": wq_t,
            "wk": wk_t,
            "wv": wv_t,
            "maskT": _mask_strip(p),
        })
    return in_maps


def _enable_ldw_opt():
    """Re-enable walrus's LDWEIGHTS optimization (defaults off in this
    toolchain); correctness is covered by the output check."""
    import concourse.bass_utils as bu
    if getattr(bu, "_ldw_patched", False):
        return
    orig = bu.run_command

    def patched(cmd, *a, **kw):
        cmd = list(cmd)
        return orig(cmd, *a, **kw)

    bu.run_command = patched
    bu._ldw_patched = True


def run(x, Wq, Wk, Wv, trace=False, trace_cores=None):
    """Returns (full_output [B,T,H] f32, BassKernelResults)."""
    from concourse.bass_utils import run_bass_kernel_spmd

    _enable_ldw_opt()

    nc = build_program()
    in_maps = make_in_maps(x, Wq, Wk, Wv)
    res = run_bass_kernel_spmd(
        nc, in_maps, list(range(N_CORES)), trace=trace,
        trace_cores=trace_cores,
    )
    full = np.empty((B, T, H), dtype=F32)
    for c in range(N_CORES):
        b, p = c // 2, c % 2
        full[b, _query_rows(p), :] = res.results[c]["out"]
    return full, res


def kernel(x, Wq, Wk, Wv):
    full, _ = run(x, Wq, Wk, Wv, trace=False)
    return full


if __name__ == "__main__":
    # quick smoke: build program only
    nc = build_program()
    print("program built ok")

